# revision 4
# baseline (speedup 1.0000x reference)
"""Cross-attention kernel for 8 Trainium2 NeuronCores.

Problem (hardcoded): B=2, NQ=NKV=2048, QDIM=KVDIM=1024, H=16, HD=64.

Sharding: tensor-parallel over heads — 2 heads per core. Each core computes
its heads' Q/K/V projections, scores, softmax and context for the full
sequence, then an AllToAll reshards context from head-split to token-split
so the output projection is fully local; core j returns output tokens
[j*512, (j+1)*512).

All matmuls run in bf16 (fp32 PSUM accumulation). Layout trick: inputs are
fed pre-transposed ([feature, token]) so every matmul operand already has
its contraction dim on partitions — the kernel contains zero on-device
transposes. scores are computed transposed ([k, q]) so the exp'd
probabilities feed the P@V matmul directly as the stationary operand, and a
ones-column appended to V yields the softmax denominator from the same
matmul (no partition-axis reduction needed).

Scheduling: projection work is chopped into ~0.5-1us chunks and drained
into the attention kt-loop as PE filler, so the in-order PE stream never
parks a whole projection tile between two attention groups (which starved
the scalar engine in the v1 kernel) and never idles waiting on exp during
attention-only stretches. Weight/input DMAs are spread across the SP /
Pool / ACT queues so the first K-projection starts as soon as possible.
"""

from collections import deque

import numpy as np
import ml_dtypes

import concourse.bass as bass
import concourse.mybir as mybir
import concourse.tile as tile
from concourse import bacc
from concourse.bass_utils import run_bass_kernel_spmd

N_CORES = 8
B = 2
NQ = NKV = 2048
C = 1024          # model dim (QDIM=KVDIM=INNER)
H, HD = 16, 64
T = B * NQ        # 4096 flattened tokens
DL = 128          # local head dims per core (2 heads * 64)
TSH = T // N_CORES  # 512 output tokens per core
SCALE = HD ** -0.5

F32 = mybir.dt.float32
BF16 = mybir.dt.bfloat16

_NC_CACHE = None
_LAST_RESULTS = None


def _build(with_collective=True, reps=None, stop_after=None):
    nc = bacc.Bacc("TRN2", target_bir_lowering=False, debug=False,
                   num_devices=N_CORES)

    qT = nc.dram_tensor("qT", [C, T], BF16, kind="ExternalInput")
    kvT = nc.dram_tensor("kvT", [C, T], BF16, kind="ExternalInput")
    wq = nc.dram_tensor("wq", [C, DL], BF16, kind="ExternalInput")
    wk = nc.dram_tensor("wk", [C, DL], BF16, kind="ExternalInput")
    wv = nc.dram_tensor("wv", [C, DL], BF16, kind="ExternalInput")
    wo = nc.dram_tensor("wo", [C, C], BF16, kind="ExternalInput")
    bias = nc.dram_tensor("bias", [C], F32, kind="ExternalInput")
    out = nc.dram_tensor("out", [TSH, C], F32, kind="ExternalOutput")

    CC = C // 128   # 8 contraction chunks
    KT = NKV // 128  # 16 k-tiles per batch
    NTILE = T // 512  # 8 projection tiles
    Exp = mybir.ActivationFunctionType.Exp

    with tile.TileContext(nc) as tc:
        with (
            tc.tile_pool(name="consts", bufs=1) as consts,
            tc.tile_pool(name="xt", bufs=3) as xt,
            tc.tile_pool(name="probs", bufs=8) as probs_p,
            tc.tile_pool(name="norm", bufs=2) as norm,
            tc.tile_pool(name="outp", bufs=2) as outp,
            tc.tile_pool(name="dram", bufs=1, space="DRAM") as dram,
        ):
            # ---- constants. wk before the x-tiles on SP; wq heads the Pool
            # queue; wv+bias ride the idle ACT queue; wo is deferred to the
            # SP queue after the last x-tile (only needed for out-proj).
            wk_sb = consts.tile([128, CC, DL], BF16)
            nc.sync.dma_start(out=wk_sb, in_=wk.ap().rearrange("(n p) d -> p n d", p=128))
            wq_sb = consts.tile([128, CC, DL], BF16)
            nc.gpsimd.dma_start(out=wq_sb, in_=wq.ap().rearrange("(n p) d -> p n d", p=128))
            wv_sb = consts.tile([128, CC, DL], BF16)
            nc.scalar.dma_start(out=wv_sb, in_=wv.ap().rearrange("(n p) d -> p n d", p=128))
            bias_sb = consts.tile([128, C], F32)
            bias_bc = bass.AP(tensor=bias, offset=0, ap=[[0, 128], [1, C]])
            nc.scalar.dma_start(out=bias_sb[:], in_=bias_bc)
            wo_sb = consts.tile([128, CC, C], BF16)

            # persistent activations
            Kd_sb = consts.tile([128, T], BF16)   # K^T: [d_local, token]
            Qd_sb = consts.tile([128, T], BF16)   # Q^T: [d_local, token]
            # V natural [token, d] in 32 tiles of [128, 130]:
            # cols 0:64 = head0, col 64 = ones, 65:129 = head1, col 129 = ones
            V_sb = consts.tile([128, T // 128, 130], BF16)
            nc.vector.memset(V_sb[:, :, 64:65], 1.0)
            nc.vector.memset(V_sb[:, :, 129:130], 1.0)

            qT_r = qT.ap().rearrange("(n p) t -> p n t", p=128)
            kvT_r = kvT.ap().rearrange("(n p) t -> p n t", p=128)

            def _body(_it=None):
                # One shared PSUM pool: tag "pss" ([128,1024] = 2 banks,
                # bufs=3) rotates scores-pair tiles, K+Q projection tiles,
                # V-projection tiles and out-proj psums; tag "psc" (1 bank,
                # bufs=2) holds the current group's two per-head context
                # accumulators.
                with tc.tile_pool(name="ps", bufs=3, space="PSUM") as ps:

                    # ---- projection work, as a deque of PE-filler chunks --
                    filler = deque()          # (tile_idx, thunk)
                    tiles_left = [0] * NTILE  # chunks still queued per tile

                    def make_chunks(tt):
                        t0 = tt * 512
                        refs = {}

                        def c_dma():
                            kvt = xt.tile([128, CC, 512], BF16, tag="kvt",
                                          name="kvt")
                            nc.sync.dma_start(out=kvt,
                                              in_=kvT_r[:, :, t0:t0 + 512])
                            qt_ = xt.tile([128, CC, 512], BF16, tag="qt",
                                          name="qt")
                            nc.gpsimd.dma_start(out=qt_,
                                                in_=qT_r[:, :, t0:t0 + 512])
                            refs["kvt"], refs["qt"] = kvt, qt_

                        def c_kq(part):
                            # part 0..3: K cc0-3, K cc4-7 + copy, Q cc0-3,
                            # Q cc4-7 + copy
                            if part == 0:
                                refs["kq"] = ps.tile([128, 1024], F32,
                                                     tag="pss", name="kq")
                            kq = refs["kq"]
                            w, x, half = (
                                (wk_sb, refs["kvt"], 0) if part < 2
                                else (wq_sb, refs["qt"], 1))
                            dst = kq[:, half * 512:(half + 1) * 512]
                            for cc in range(4 * (part % 2), 4 * (part % 2) + 4):
                                nc.tensor.matmul(dst, lhsT=w[:, cc, :],
                                                 rhs=x[:, cc, :],
                                                 start=(cc == 0), stop=(cc == CC - 1))
                            if part % 2 == 1:
                                dsb = Kd_sb if part == 1 else Qd_sb
                                nc.vector.tensor_copy(
                                    out=dsb[:, t0:t0 + 512], in_=dst)

                        def c_v(s4):
                            if s4 == 0:
                                refs["pv"] = ps.tile([128, 1024], F32,
                                                     tag="pss", name="pvp")
                            pvt = refs["pv"]
                            dst = pvt[:, s4 * 128:(s4 + 1) * 128]
                            for cc in range(CC):
                                nc.tensor.matmul(
                                    dst,
                                    lhsT=refs["kvt"][:, cc, s4 * 128:(s4 + 1) * 128],
                                    rhs=wv_sb[:, cc, :],
                                    start=(cc == 0), stop=(cc == CC - 1))
                            ti = tt * 4 + s4
                            # one strided copy fills both head blocks
                            # (cols 0:64 and 65:129), skipping the ones
                            # columns: out free pattern [2 (stride 65), 64]
                            vdst = V_sb[:, ti, 0:64]
                            vdst2 = bass.AP(
                                tensor=vdst.tensor, offset=vdst.offset,
                                ap=[vdst.ap[0], [65, 2], [1, 64]])
                            nc.vector.tensor_copy(
                                out=vdst2,
                                in_=dst.rearrange("p (g x) -> p g x", g=2))

                        chunks = [c_dma]
                        chunks += [lambda p=p: c_kq(p) for p in range(4)]
                        chunks += [lambda s=s: c_v(s) for s in range(4)]
                        return chunks

                    for tt in range(NTILE):
                        cs = make_chunks(tt)
                        tiles_left[tt] = len(cs)
                        for c in cs:
                            filler.append((tt, c))

                    def drain(k):
                        for _ in range(k):
                            if not filler:
                                return
                            tt, c = filler.popleft()
                            c()
                            tiles_left[tt] -= 1

                    def ensure_tile(tt):
                        while tiles_left[tt] > 0:
                            drain(1)

                    # deferred wo load on SP, after the last x-tile DMA
                    wo_loaded = [False]

                    def load_wo():
                        if not wo_loaded[0]:
                            wo_loaded[0] = True
                            nc.sync.dma_start(
                                out=wo_sb,
                                in_=wo.ap().rearrange("(n p) e -> p n e", p=128))

                    # ---- attention groups ----
                    a2a_in = dram.tile([N_CORES, DL, TSH], BF16)
                    a2a_out = dram.tile([N_CORES, DL, TSH], BF16)

                    def scores(b, q0, kt):
                        k0 = b * NKV + kt * 128
                        # both heads' transposed scores into one 2-bank
                        # tile -> a single wide exp
                        pair = ps.tile([128, 1024], F32, tag="pss", name="pair")
                        for h in range(2):
                            hs = slice(h * 64, (h + 1) * 64)
                            nc.tensor.matmul(
                                pair[:, h * 512:(h + 1) * 512],
                                lhsT=Kd_sb[hs, k0:k0 + 128],
                                rhs=Qd_sb[hs, q0:q0 + 512],
                                start=True, stop=True)
                        return pair

                    def pv(b, psc, kt, pr):
                        vt = b * KT + kt
                        for h in range(2):
                            nc.tensor.matmul(
                                psc[h],
                                lhsT=V_sb[:, vt, h * 65:(h + 1) * 65],
                                rhs=pr[:, h * 512:(h + 1) * 512],
                                start=(kt == 0), stop=(kt == KT - 1))

                    def attn_norm(b, qv, psc):
                        q0 = b * NQ + qv * 512
                        j = q0 // TSH
                        for h in range(2):
                            recip = norm.tile([1, 512], F32, tag="recip",
                                              name="recip")
                            nc.vector.reciprocal(out=recip, in_=psc[h][64:65, :])
                            bc = norm.tile([64, 512], F32, tag="bc", name="bc")
                            nc.gpsimd.partition_broadcast(bc[:], recip[:])
                            ctxn = norm.tile([64, 512], BF16, tag="ctxn",
                                             name="ctxn")
                            nc.vector.tensor_mul(ctxn, psc[h][0:64, :], bc)
                            nc.sync.dma_start(
                                out=a2a_in[j, h * 64:(h + 1) * 64, :], in_=ctxn)

                    groups = [(b, qv) for b in range(B) for qv in range(4)]
                    for b, qv in groups:
                        q0 = b * NQ + qv * 512
                        ensure_tile(b * 4 + qv)  # Q source for this group
                        ensure_tile(b * 4)       # K/V for kt 0-3
                        psc = [ps.tile([65, 512], F32, tag="psc",
                                       name=f"psc{h}", bufs=2)
                               for h in range(2)]
                        # Software-pipelined emission: scores(kt+1) is
                        # placed BEFORE pv(kt) in the (in-order) PE stream,
                        # so the PE never stalls on exp(kt) before issuing
                        # the next scores pair; proj filler chunks are
                        # drained AFTER pv so they fill the PE while ACT
                        # grinds exps.
                        pair = scores(b, q0, 0)
                        for n in range(KT):
                            pr = probs_p.tile([128, 1024], BF16, tag="probs",
                                              name="pr")
                            nc.scalar.activation(out=pr, in_=pair, func=Exp,
                                                 scale=SCALE)
                            if n + 1 < KT:
                                ensure_tile(b * 4 + (n + 1) // 4)
                                pair = scores(b, q0, n + 1)
                            pv(b, psc, n, pr)
                            drain(2 if b == 0 else 1)
                            if not filler:
                                load_wo()
                        attn_norm(b, qv, psc)
                    load_wo()

                    if stop_after == "attn":
                        return
                    if with_collective:
                        nc.gpsimd.collective_compute(
                            "AllToAll", mybir.AluOpType.bypass,
                            replica_groups=[list(range(N_CORES))],
                            ins=[a2a_in.opt()], outs=[a2a_out.opt()])
                    else:
                        a2a_out = a2a_in  # timing-sim variant: skip collective

                    # ---- output projection (local tokens only) ----
                    ctxF = outp.tile([128, N_CORES, TSH], BF16)
                    for i in range(N_CORES):
                        nc.sync.dma_start(out=ctxF[:, i, :], in_=a2a_out[i])
                    for m in range(TSH // 128):
                        ob = outp.tile([128, C], F32, tag="ob", name="ob")
                        pso = ps.tile([128, 1024], F32, tag="pss", name="pso")
                        for half in range(2):
                            for i in range(N_CORES):
                                nc.tensor.matmul(
                                    pso[:, half * 512:(half + 1) * 512],
                                    lhsT=ctxF[:, i, m * 128:(m + 1) * 128],
                                    rhs=wo_sb[:, i, half * 512:(half + 1) * 512],
                                    start=(i == 0), stop=(i == N_CORES - 1))
                            nc.vector.tensor_add(
                                ob[:, half * 512:(half + 1) * 512],
                                pso[:, half * 512:(half + 1) * 512],
                                bias_sb[:, half * 512:(half + 1) * 512])
                        nc.sync.dma_start(out=out.ap()[m * 128:(m + 1) * 128, :],
                                          in_=ob)

            if reps is None:
                _body()
            else:
                with tc.For_i(0, reps, 1) as _it:
                    _body(_it)
    nc.compile()
    return nc


def _get_nc():
    global _NC_CACHE
    if _NC_CACHE is None:
        _NC_CACHE = _build()
    return _NC_CACHE


def prep_in_maps(query, key_value, w_q, w_kv, w_out, b_out):
    bf = ml_dtypes.bfloat16
    q2 = np.asarray(query, np.float32).reshape(T, C)
    kv2 = np.asarray(key_value, np.float32).reshape(T, C)
    qT = np.ascontiguousarray(q2.T).astype(bf)
    kvT = np.ascontiguousarray(kv2.T).astype(bf)
    wo = np.asarray(w_out, np.float32).astype(bf)
    bias = np.asarray(b_out, np.float32)

    in_maps = []
    for j in range(N_CORES):
        cs = slice(j * DL, (j + 1) * DL)
        in_maps.append({
            "qT": qT,
            "kvT": kvT,
            "wq": np.ascontiguousarray(np.asarray(w_q, np.float32)[:, cs]).astype(bf),
            "wk": np.ascontiguousarray(np.asarray(w_kv, np.float32)[:, cs]).astype(bf),
            "wv": np.ascontiguousarray(
                np.asarray(w_kv, np.float32)[:, C + j * DL: C + (j + 1) * DL]).astype(bf),
            "wo": wo,
            "bias": bias,
        })
    return in_maps


def kernel(query, key_value, w_q, w_kv, w_out, b_out):
    global _LAST_RESULTS
    in_maps = prep_in_maps(query, key_value, w_q, w_kv, w_out, b_out)
    nc = _get_nc()
    res = run_bass_kernel_spmd(nc, in_maps, core_ids=list(range(N_CORES)))
    _LAST_RESULTS = res
    full = np.concatenate([res.results[j]["out"] for j in range(N_CORES)], axis=0)
    return full.reshape(B, NQ, C)
